# revision 2
# baseline (speedup 1.0000x reference)
"""Trainium2 Bass kernel for nn_Greedy_Base_hGLM.

Data-parallel over time T across 8 NeuronCores, no collectives: the
causal-conv halo is handled by overlapping input shards (256 extra
timesteps per core).

Per core:
  host:  REBAR reparam math on tiny [20,2500] params -> 3 C_syn variants;
         double-exp conv kernels -> block-Toeplitz lhsT matrices;
         shard S_e/S_i over T with halo, transpose to feature-major bf16
         (matmul contracts the partition dim; DMA-transpose is
         2-byte-only, so the layout is prepared host-side while sharding).
  device: projection matmuls (C_all stationary, S^T moving) -> in_e/in_i
         for all 3 variants at once; PE-transpose back to time-major;
         201-tap causal depthwise conv as block-Toeplitz matmuls;
         per-timestep tree-tanh recurrence (DVE/ACT); V for 3 variants.
  host:  reassemble [3, 20000]; small outputs (theta, hard_z, soft_z,
         soft_zb) from the host math.
"""

import numpy as np
import ml_dtypes

SUB_NO = 20
T_NO = 201
E_NO = 2000
I_NO = 500
T_DATA = 20000

N_CORES = 8
T_PER = 2500          # output timesteps owned per core
BLK = 128             # time block (= partition dim)
OUT_BLKS = 20         # ceil(2500/128) output blocks per core
HALO_BLKS = 2         # 256-step halo covers the 201-tap causal kernel
IN_BLKS = OUT_BLKS + HALO_BLKS          # 22
T_IN = IN_BLKS * BLK                    # 2816 input timesteps per core
HALO = HALO_BLKS * BLK                  # 256

E_PAD = 2048
I_PAD = 512
J_ROWS = E_PAD + I_PAD                  # 2560
J_TILES = J_ROWS // 128                 # 20
N_CH = 120            # (e/i) * 20 subunits * 3 variants
N_V = 3               # hard, soft, soft_b

BF16 = ml_dtypes.bfloat16


def _host_small_math(u, v, C_syn_log):
    """REBAR reparam: theta, hard_z, soft_z, soft_zb (all float32)."""
    x = C_syn_log - C_syn_log.max(axis=0, keepdims=True)
    ex = np.exp(x)
    theta = (ex / ex.sum(axis=0, keepdims=True)).astype(np.float32)
    rebar_z = np.log(theta) - np.log(-np.log(u))
    idx = np.argmax(rebar_z, axis=0)
    hard_z = np.zeros_like(rebar_z)
    hard_z[idx, np.arange(rebar_z.shape[1])] = 1.0
    v_k = np.sum(v * hard_z, axis=0, keepdims=True)
    z_same = -np.log(-np.log(v))
    z_diff = -np.log(-np.log(v) / theta - np.log(v_k))
    rebar_zb = hard_z * z_same + (1.0 - hard_z) * z_diff
    sig = lambda t: (1.0 / (1.0 + np.exp(-t / np.float32(0.5)))).astype(np.float32)
    soft_z = sig(rebar_z) + np.float32(1e-9)
    soft_zb = sig(rebar_zb) + np.float32(1e-9)
    return theta, hard_z.astype(np.float32), soft_z, soft_zb


def _conv_kernels(W_syn, Tau_syn, Delta_syn):
    """Double-exponential synaptic kernels kern[s, ei, tau]  (float32)."""
    t_raw = np.arange(T_NO, dtype=np.float32)
    t = np.maximum(t_raw[None, None, :] - np.exp(Delta_syn)[:, :, None], 0.0)
    t_tau = t / np.exp(Tau_syn)[:, :, None]
    return (t_tau * np.exp(-t_tau) * W_syn[:, :, None]).astype(np.float32)


def _toeplitz_lhsT(kern):
    """Causal conv as block matmuls over 128-step blocks:

      y[i+128b] = sum_d sum_j Td[i,j] x[j+128(b-d)],  Td[i,j]=kern[i-j+128d]

    Returns lhsT (= Td transposed, [j, i]) per live d; d's whose block is
    negligible (kernel support shorter than 128d) are dropped.
    """
    i = np.arange(BLK)[None, :]
    j = np.arange(BLK)[:, None]
    mats = {}
    gmax = np.abs(kern).max() + 1e-30
    d_list = []
    for d in range(3):
        tau = i - j + BLK * d                     # [j, i]
        mask = (tau >= 0) & (tau < T_NO)
        tauc = np.clip(tau, 0, T_NO - 1)
        md = kern[:, :, tauc] * mask[None, None]  # [s, ei, j, i]
        if np.abs(md).max() > 1e-7 * gmax:
            d_list.append(d)
            mats[d] = md.astype(np.float32)
    return mats, d_list


def _build_program(nd, d_list, w_sub, vo):
    """Trace + compile the SPMD Bass program (one NEFF, all 8 cores).

    w_sub / vo are python floats baked into the instruction stream (same
    values for every core, so still SPMD-safe).
    """
    import concourse.bacc as bacc
    import concourse.mybir as mybir
    from concourse import bass, tile

    f32 = mybir.dt.float32
    bf16 = mybir.dt.bfloat16
    Act = mybir.ActivationFunctionType
    Alu = mybir.AluOpType

    nc = bacc.Bacc("TRN2", target_bir_lowering=False, debug=False,
                   num_devices=N_CORES)

    sT_d = nc.dram_tensor("sT", [J_ROWS, T_IN], bf16, kind="ExternalInput")
    c_d = nc.dram_tensor("c_all", [128, J_TILES * N_CH], bf16,
                         kind="ExternalInput")
    t_d = nc.dram_tensor("toep", [128, nd * 2 * SUB_NO * 128], bf16,
                         kind="ExternalInput")
    id_d = nc.dram_tensor("ident", [N_CH, N_CH], f32, kind="ExternalInput")
    v_d = nc.dram_tensor("v_out", [BLK, OUT_BLKS * N_V], f32,
                         kind="ExternalOutput")

    CHUNKS = []
    off = 0
    while off < T_IN:
        w = min(512, T_IN - off)
        CHUNKS.append((off, w))
        off += w

    with tile.TileContext(nc) as tc:
        with (
            tc.tile_pool(name="const", bufs=1) as cpool,
            tc.tile_pool(name="sres", bufs=1) as spool,
            tc.tile_pool(name="pei", bufs=1) as peipool,
            tc.tile_pool(name="xall", bufs=1) as xpool,
            tc.tile_pool(name="sub", bufs=1) as subpool,
            tc.tile_pool(name="tmp", bufs=3) as tmppool,
            tc.tile_pool(name="pproj", bufs=2, space=bass.MemorySpace.PSUM) as ppsum,
            tc.tile_pool(name="ptrans", bufs=2, space=bass.MemorySpace.PSUM) as tpsum,
            tc.tile_pool(name="pconv", bufs=4, space=bass.MemorySpace.PSUM) as kpsum,
        ):
            c_sb = cpool.tile([128, J_TILES, N_CH], bf16, tag="c")
            nc.sync.dma_start(c_sb[:], c_d[:].rearrange("p (j c) -> p j c",
                                                        j=J_TILES))
            t_sb = cpool.tile([128, nd * 2 * SUB_NO, 128], bf16, tag="t")
            nc.sync.dma_start(t_sb[:], t_d[:].rearrange("p (m i) -> p m i",
                                                        m=nd * 2 * SUB_NO))
            ident = cpool.tile([N_CH, N_CH], f32, tag="id")
            nc.sync.dma_start(ident[:], id_d[:])

            s_tiles = []
            for j in range(J_TILES):
                st = spool.tile([128, T_IN], bf16, tag=f"s{j}")
                nc.sync.dma_start(st[:], sT_d[j * 128:(j + 1) * 128, :])
                s_tiles.append(st)

            # conv input, time-major bf16: [t_in_block, block, channel]
            x3 = xpool.tile([BLK, IN_BLKS, N_CH], bf16, tag="x")

            # ---- projection (contract j), then PE-transpose to time-major
            for ci, (off, w) in enumerate(CHUNKS):
                ps = ppsum.tile([N_CH, 512], f32, tag="proj")
                for j in range(J_TILES):
                    nc.tensor.matmul(
                        ps[:, :w], c_sb[:, j, :], s_tiles[j][:, off:off + w],
                        start=(j == 0), stop=(j == J_TILES - 1))
                pc = peipool.tile([N_CH, 512], f32, tag=f"pei{ci}")
                nc.vector.tensor_copy(pc[:, :w], ps[:, :w])
                for bb in range(w // BLK):
                    tp = tpsum.tile([BLK, N_CH], f32, tag="tp")
                    nc.tensor.transpose(
                        tp[:], pc[:, bb * BLK:(bb + 1) * BLK], ident[:])
                    nc.vector.tensor_copy(x3[:, off // BLK + bb, :], tp[:])

            # ---- conv (block-Toeplitz matmuls) + tree recurrence
            sub = [None] * SUB_NO
            for s in range(SUB_NO - 1, -1, -1):
                cp = kpsum.tile([BLK, OUT_BLKS, N_V], f32, tag="conv")
                n_mm = 2 * nd
                k = 0
                for ei in range(2):
                    ch0 = ei * 60 + s * N_V
                    for di in range(nd):
                        m = (ei * nd + di) * SUB_NO + s
                        d = d_list[di]
                        nc.tensor.matmul(
                            cp[:],
                            t_sb[:, m, :],
                            x3[:, HALO_BLKS - d:IN_BLKS - d, ch0:ch0 + N_V],
                            start=(k == 0), stop=(k == n_mm - 1))
                        k += 1
                so = subpool.tile([BLK, OUT_BLKS * N_V], f32, tag=f"sub{s}")
                acc = cp[:].rearrange("p b v -> p (b v)")
                kids = [c for c in (2 * s + 1, 2 * s + 2) if c < SUB_NO]
                for c in kids:
                    tt = tmppool.tile([BLK, OUT_BLKS * N_V], f32, tag="tmp")
                    nc.vector.scalar_tensor_tensor(
                        tt[:], sub[c][:], float(w_sub[c]), acc,
                        op0=Alu.mult, op1=Alu.add)
                    acc = tt[:]
                nc.scalar.activation(so[:], acc, Act.Tanh)
                sub[s] = so

            vt = tmppool.tile([BLK, OUT_BLKS * N_V], f32, tag="vout")
            nc.vector.tensor_scalar(
                vt[:], sub[0][:], float(w_sub[0]), float(vo),
                Alu.mult, Alu.add)
            nc.sync.dma_start(v_d[:], vt[:])

    nc.compile()
    return nc


def kernel(S_e, S_i, u, v, W_syn, Tau_syn, Delta_syn, W_sub, V_o, Theta,
           C_syn_log):
    theta, hard_z, soft_z, soft_zb = _host_small_math(
        np.asarray(u, np.float32), np.asarray(v, np.float32),
        np.asarray(C_syn_log, np.float32))

    kern = _conv_kernels(np.asarray(W_syn, np.float32),
                         np.asarray(Tau_syn, np.float32),
                         np.asarray(Delta_syn, np.float32))
    mats, d_list = _toeplitz_lhsT(kern)
    nd = len(d_list)

    # ---- projection weights: [j, ch], ch = ei*60 + s*3 + v
    variants = (hard_z, soft_z, soft_zb)
    C_all = np.zeros((J_ROWS, N_CH), np.float32)
    for vi, cz in enumerate(variants):
        C_all[:E_NO, 0 * 60 + np.arange(SUB_NO) * N_V + vi] = cz[:, :E_NO].T
        C_all[E_PAD:E_PAD + I_NO, 60 + np.arange(SUB_NO) * N_V + vi] = \
            cz[:, E_NO:].T
    c_dev = np.ascontiguousarray(
        C_all.reshape(J_TILES, 128, N_CH).transpose(1, 0, 2)
    ).astype(BF16).reshape(128, J_TILES * N_CH)

    # ---- Toeplitz lhsT upload: [j, m, i], m = (ei*nd + di)*20 + s
    t_dev = np.zeros((128, nd * 2 * SUB_NO, 128), np.float32)
    for di, d in enumerate(d_list):
        for ei in range(2):
            for s in range(SUB_NO):
                t_dev[:, (ei * nd + di) * SUB_NO + s, :] = mats[d][s, ei]
    t_dev = t_dev.astype(BF16).reshape(128, -1)

    ident = np.eye(N_CH, dtype=np.float32)

    # ---- shard S over T (halo + tail padding via one zero-padded transpose)
    width = HALO + (N_CORES - 1) * T_PER + T_IN
    SeT = np.zeros((E_PAD, width), BF16)
    SeT[:E_NO, HALO:HALO + T_DATA] = np.asarray(S_e).astype(BF16).T
    SiT = np.zeros((I_PAD, width), BF16)
    SiT[:I_NO, HALO:HALO + T_DATA] = np.asarray(S_i).astype(BF16).T

    in_maps = []
    for c in range(N_CORES):
        lo = c * T_PER
        sT = np.empty((J_ROWS, T_IN), BF16)
        sT[:E_PAD] = SeT[:, lo:lo + T_IN]
        sT[E_PAD:] = SiT[:, lo:lo + T_IN]
        in_maps.append({
            "sT": sT, "c_all": c_dev, "toep": t_dev, "ident": ident,
        })

    nc = _build_program(nd, d_list, np.asarray(W_sub, np.float64),
                        float(np.asarray(V_o).reshape(-1)[0]))

    from concourse.bass_utils import run_bass_kernel_spmd
    res = run_bass_kernel_spmd(nc, in_maps, list(range(N_CORES)))

    V = np.empty((N_V, T_DATA), np.float32)
    for c in range(N_CORES):
        arr = np.asarray(res.results[c]["v_out"], np.float32)   # [128, 60]
        flat = arr.reshape(BLK, OUT_BLKS, N_V).transpose(1, 0, 2) \
                  .reshape(OUT_BLKS * BLK, N_V)
        V[:, c * T_PER:(c + 1) * T_PER] = flat[:T_PER].T

    return (V[0], V[1], V[2], theta, hard_z, soft_z, soft_zb)
